# revision 1
# baseline (speedup 1.0000x reference)
"""GCN message-passing layer on 8 Trainium2 NeuronCores.

out = segment_sum(x[src], dst, N) @ W.T + b

Design (per core, dst-sharded, fp16 datapath):
  - Host: capacity-constrained greedy pack of dst nodes into 8 cores x 40
    tiles x 128 lanes so every tile has exactly cap_a A-half + cap_b B-half
    128-edge chunks (A/B split at row 32768 for int16 gather indices). x and
    W are cast to fp16 on host; edge lists become gather-index tables and a
    per-chunk dst-lane table (dl).
  - Device: 1024-idx dma_gather calls pull fp16 x rows (256B descriptors,
    half the fp32 bytes) into [128 edge, 128 feat] chunk tiles grouped 8
    tiles per buffer; DVE tensor_scalar(is_equal) builds each chunk's one-hot
    scatter matrix; PE matmuls (fp16, 1 cyc/row, 4x faster than fp32)
    scatter-accumulate h^T per tile into PSUM (4 tiles per 2KB bank tile);
    Activation flushes PSUM->SBUF fp16; PE applies W^T per tile and the bias
    rides the Activation copy (per-partition bias add); results are written
    feature-major as fp16 (1KB contiguous rows for full DMA bandwidth).
  - Host: inverse-permute per-core fp16 outputs to node order, cast fp32.
"""

import sys

import numpy as np

sys.path.insert(0, "/opt/trn_rl_repo")

N_NODES = 40000
N_EDGES = 640000
D = 128
P = 128
N_CORES = 8
TILES_PER_CORE = 40
N_BINS = N_CORES * TILES_PER_CORE  # 320 tiles of up to 128 nodes
SLOTS_PER_CORE = TILES_PER_CORE * P  # 5120
HALF = 32768  # int16 gather index limit; A half = [0, HALF), B = [HALF, N)
NB = N_NODES - HALF
TPG = 8  # tiles per gather group
GROUPS = TILES_PER_CORE // TPG
PAD_DL = 1000.0  # dl sentinel for pad slots -> all-zero one-hot column
SCRATCH = 16384  # SWDGE ring bytes/partition
MAX_CALL = 1024

_PROGRAM_CACHE: dict = {}


def _pack_nodes(degA, degB, capA, capB):
    """Greedy pack nodes into N_BINS bins: <=128 nodes, <=capA*128 A-edges,
    <=capB*128 B-edges per bin. Returns (node_bin, node_lane) or None."""
    import heapq

    limA, limB = capA * P, capB * P
    deg = degA + degB
    order = np.argsort(-deg, kind="stable")
    heap = [(0.0, b) for b in range(N_BINS)]
    heapq.heapify(heap)
    cnt = np.zeros(N_BINS, dtype=np.int64)
    ldA = np.zeros(N_BINS, dtype=np.int64)
    ldB = np.zeros(N_BINS, dtype=np.int64)
    node_bin = np.empty(N_NODES, dtype=np.int32)
    node_lane = np.empty(N_NODES, dtype=np.int32)
    for n in order:
        dA, dB = int(degA[n]), int(degB[n])
        stash = []
        placed = False
        while heap:
            s, b = heapq.heappop(heap)
            if cnt[b] < P and ldA[b] + dA <= limA and ldB[b] + dB <= limB:
                node_bin[n] = b
                node_lane[n] = cnt[b]
                cnt[b] += 1
                ldA[b] += dA
                ldB[b] += dB
                ns = max(
                    (ldA[b] + dA) / limA, (ldB[b] + dB) / limB, (cnt[b] + 1) / P
                )
                stash.append((ns, b))
                placed = True
                break
            if cnt[b] < P:
                stash.append((s + 0.02, b))  # penalize cap-blocked bins
        for item in stash:
            heapq.heappush(heap, item)
        if not placed:
            return None
    return node_bin, node_lane


def _wrap_idx(arr: np.ndarray) -> np.ndarray:
    """int16 flat idx list -> [128, len/16] wrapped + replicated layout."""
    w = arr.reshape(-1, 16).T  # [16, n/16]
    return np.ascontiguousarray(np.tile(w, (8, 1)))


def _prepare(x, src, dst, W, b):
    src = np.asarray(src).astype(np.int64)
    dst = np.asarray(dst).astype(np.int64)
    isB = src >= HALF
    degA = np.bincount(dst[~isB], minlength=N_NODES)
    degB = np.bincount(dst[isB], minlength=N_NODES)

    packed = None
    for capA, capB in ((13, 3), (13, 4), (14, 3), (14, 4), (15, 5)):
        packed = _pack_nodes(degA, degB, capA, capB)
        if packed is not None:
            break
    assert packed is not None, "packing failed"
    node_bin, node_lane = packed
    K = capA + capB

    ebin = node_bin[dst]
    elane = node_lane[dst].astype(np.int64)
    eorder = np.lexsort((src, isB, ebin))
    s_src, s_half, s_bin, s_lane = (
        src[eorder],
        isB[eorder].astype(np.int64),
        ebin[eorder],
        elane[eorder],
    )

    key = s_bin * 2 + s_half
    seg_start = np.searchsorted(key, np.arange(N_BINS * 2), side="left")
    seg_end = np.searchsorted(key, np.arange(N_BINS * 2), side="right")

    nA_call = TPG * capA * P  # idxs per A gather group
    nB_call = TPG * capB * P
    idxA = np.zeros((N_CORES, TILES_PER_CORE, capA * P), dtype=np.int16)
    idxB = np.zeros((N_CORES, TILES_PER_CORE, capB * P), dtype=np.int16)
    # dl4[core, tile, j, p] = dst lane of edge at chunk j, position p
    dl4 = np.full((N_CORES, TILES_PER_CORE, K, P), PAD_DL, dtype=np.float32)
    for c in range(N_CORES):
        for t in range(TILES_PER_CORE):
            g = (c * TILES_PER_CORE + t) * 2
            a0, a1 = seg_start[g], seg_end[g]
            nA = a1 - a0
            idxA[c, t, :nA] = s_src[a0:a1].astype(np.int16)
            dl4[c, t, :capA].reshape(-1)[:nA] = s_lane[a0:a1]
            b0, b1 = seg_start[g + 1], seg_end[g + 1]
            nB = b1 - b0
            idxB[c, t, :nB] = (s_src[b0:b1] - HALF).astype(np.int16)
            dl4[c, t, capA:].reshape(-1)[:nB] = s_lane[b0:b1]
    # device layout: dl[core, p, tile*K + j]
    dl = np.ascontiguousarray(dl4.transpose(0, 3, 1, 2).reshape(N_CORES, P, -1))

    iota = np.tile(np.arange(P, dtype=np.float16)[None, :], (P, 1))
    wt = np.ascontiguousarray(np.asarray(W).T.astype(np.float16))
    bc = np.asarray(b).astype(np.float32)[:, None]  # [128,1] act-bias column
    xa = np.ascontiguousarray(np.asarray(x)[:HALF].astype(np.float16))
    xb = np.ascontiguousarray(np.asarray(x)[HALF:].astype(np.float16))

    in_maps = []
    for c in range(N_CORES):
        in_maps.append(
            {
                "xa": xa,
                "xb": xb,
                "idxa": _wrap_idx(idxA[c].reshape(-1)),
                "idxb": _wrap_idx(idxB[c].reshape(-1)),
                "dl": np.ascontiguousarray(dl[c]),
                "iota": iota,
                "wt": wt,
                "bc": bc,
            }
        )

    slot_node = np.full(N_BINS * P, -1, dtype=np.int64)
    slot_node[node_bin.astype(np.int64) * P + node_lane] = np.arange(N_NODES)
    return in_maps, (capA, capB), slot_node


def _build_program(capA: int, capB: int):
    import concourse.mybir as mybir
    import concourse.tile as tile
    from concourse import bacc

    K = capA + capB
    f16 = mybir.dt.float16
    f32 = mybir.dt.float32
    nA_call = TPG * capA * P
    nB_call = TPG * capB * P

    def subcalls(n):
        out = []
        while n > 0:
            s = min(MAX_CALL, n)
            out.append(s)
            n -= s
        return out

    nc = bacc.Bacc("TRN2", dynamic_dma_scratch_size=SCRATCH)
    xa = nc.dram_tensor("xa", [HALF, D], f16, kind="ExternalInput")
    xb = nc.dram_tensor("xb", [NB, D], f16, kind="ExternalInput")
    idxa = nc.dram_tensor(
        "idxa", [P, GROUPS * nA_call // 16], mybir.dt.int16, kind="ExternalInput"
    )
    idxb = nc.dram_tensor(
        "idxb", [P, GROUPS * nB_call // 16], mybir.dt.int16, kind="ExternalInput"
    )
    dl_d = nc.dram_tensor("dl", [P, TILES_PER_CORE * K], f32, kind="ExternalInput")
    iota_d = nc.dram_tensor("iota", [P, P], f16, kind="ExternalInput")
    wt_d = nc.dram_tensor("wt", [D, D], f16, kind="ExternalInput")
    bc_d = nc.dram_tensor("bc", [D, 1], f32, kind="ExternalInput")
    outT = nc.dram_tensor("outT", [P, SLOTS_PER_CORE], f16, kind="ExternalOutput")

    with tile.TileContext(nc) as tc:
        with (
            tc.tile_pool(name="const", bufs=1) as cpool,
            tc.tile_pool(name="ma", bufs=3) as ma_pool,
            tc.tile_pool(name="mb", bufs=2) as mb_pool,
            tc.tile_pool(name="pp", bufs=4) as pp_pool,
            tc.tile_pool(name="hsb", bufs=2) as h_pool,
            tc.tile_pool(name="osb", bufs=2) as o_pool,
            tc.tile_pool(name="psh", bufs=2, space="PSUM") as psh_pool,
            tc.tile_pool(name="pso", bufs=2, space="PSUM") as pso_pool,
        ):
            idxa_t = cpool.tile([P, GROUPS * nA_call // 16], mybir.dt.int16)
            nc.sync.dma_start(out=idxa_t[:], in_=idxa[:])
            idxb_t = cpool.tile([P, GROUPS * nB_call // 16], mybir.dt.int16)
            nc.sync.dma_start(out=idxb_t[:], in_=idxb[:])
            dl_t = cpool.tile([P, TILES_PER_CORE * K], f32)
            nc.sync.dma_start(out=dl_t[:], in_=dl_d[:])
            iota_t = cpool.tile([P, P], f16)
            nc.sync.dma_start(out=iota_t[:], in_=iota_d[:])
            wt_t = cpool.tile([D, D], f16)
            nc.sync.dma_start(out=wt_t[:], in_=wt_d[:])
            bc_t = cpool.tile([D, 1], f32)
            nc.sync.dma_start(out=bc_t[:], in_=bc_d[:])

            for g in range(GROUPS):
                ma = ma_pool.tile([P, TPG * capA, D], f16, tag="ma")
                off = 0
                for sub in subcalls(nA_call):
                    o0 = (g * nA_call + off) // 16
                    nc.gpsimd.dma_gather(
                        out_ap=ma[:, off // P : (off + sub) // P, :],
                        in_ap=xa[:],
                        idxs_ap=idxa_t[:, o0 : o0 + sub // 16],
                        num_idxs=sub,
                        num_idxs_reg=sub,
                        elem_size=D,
                        elem_step=D,
                    )
                    off += sub
                mb = mb_pool.tile([P, TPG * capB, D], f16, tag="mb")
                off = 0
                for sub in subcalls(nB_call):
                    o0 = (g * nB_call + off) // 16
                    nc.gpsimd.dma_gather(
                        out_ap=mb[:, off // P : (off + sub) // P, :],
                        in_ap=xb[:],
                        idxs_ap=idxb_t[:, o0 : o0 + sub // 16],
                        num_idxs=sub,
                        num_idxs_reg=sub,
                        elem_size=D,
                        elem_step=D,
                    )
                    off += sub
                for tig in range(TPG):
                    t = g * TPG + tig
                    pt = pp_pool.tile([P, K, P], f16, tag="pp")
                    for j in range(K):
                        nc.vector.tensor_scalar(
                            out=pt[:, j, :], in0=iota_t[:],
                            scalar1=dl_t[:, t * K + j : t * K + j + 1],
                            scalar2=None, op0=mybir.AluOpType.is_equal,
                        )
                    q = t % 4
                    if q == 0:
                        ps = psh_pool.tile([P, 4 * P], f32, tag="psh")
                    for j in range(capA):
                        nc.tensor.matmul(
                            out=ps[:, q * P : (q + 1) * P],
                            lhsT=ma[:, tig * capA + j, :],
                            rhs=pt[:, j, :],
                            start=(j == 0),
                            stop=False,
                        )
                    for j in range(capB):
                        nc.tensor.matmul(
                            out=ps[:, q * P : (q + 1) * P],
                            lhsT=mb[:, tig * capB + j, :],
                            rhs=pt[:, capA + j, :],
                            start=False,
                            stop=(j == capB - 1),
                        )
                    if q == 3:
                        hsb = h_pool.tile([P, 4 * P], f16, tag="hsb")
                        nc.scalar.copy(out=hsb[:], in_=ps[:])
                        po = pso_pool.tile([P, 4 * P], f32, tag="pso")
                        for k in range(4):
                            nc.tensor.matmul(
                                out=po[:, k * P : (k + 1) * P],
                                lhsT=wt_t[:],
                                rhs=hsb[:, k * P : (k + 1) * P],
                                start=True,
                                stop=True,
                            )
                        osb = o_pool.tile([P, 4 * P], f16, tag="osb")
                        nc.scalar.add(out=osb[:], in_=po[:], add=bc_t[:])
                        nc.sync.dma_start(
                            out=outT[:, (t - 3) * P : (t + 1) * P], in_=osb[:]
                        )

    nc.finalize()
    return nc


def get_program(capA: int, capB: int):
    key = (capA, capB)
    if key not in _PROGRAM_CACHE:
        _PROGRAM_CACHE[key] = _build_program(capA, capB)
    return _PROGRAM_CACHE[key]


def kernel(x, src, dst, W, b):
    from concourse.bass_utils import run_bass_kernel_spmd

    in_maps, caps, slot_node = _prepare(x, src, dst, W, b)
    nc = get_program(*caps)
    res = run_bass_kernel_spmd(nc, in_maps, list(range(N_CORES)))

    full = np.empty((N_NODES, D), dtype=np.float32)
    for c in range(N_CORES):
        o = res.results[c]["outT"]  # [128 feat, 5120 slots] f16
        sn = slot_node[c * SLOTS_PER_CORE : (c + 1) * SLOTS_PER_CORE]
        valid = sn >= 0
        full[sn[valid]] = o[:, valid].T.astype(np.float32)
    return full



# revision 19
# speedup vs baseline: 1.2447x; 1.2447x over previous
"""GCN message-passing layer on 8 Trainium2 NeuronCores.

out = segment_sum(x[src], dst, N) @ W.T + b

Design (per core, dst-sharded, narrow 16-lane tiles):
  - Host: nodes packed into 8 cores x 320 tiles x 16 lanes; each tile has 2
    fixed 128-edge chunks (A: src<32768, B: src>=16384 via a base-offset view
    of one u64 x table; the overlap zone [16384,32768) balances the halves).
    Scatter one-hots ([128 slot, 16 lane] f16 per chunk) are host-built
    structural tables (dst indices only) and DMA-loaded, not computed on DVE.
  - Device: x rows are gathered as 32xuint64 elements (256B rows; the SWDGE
    descgen cost model charges per element, so u64 is 4x cheaper than f16);
    PE scatter-accumulates h^T per tile into PSUM via [128x128]@[128x16]
    matmuls against the preloaded one-hots; DVE flushes PSUM to fp16; one
    [128x512] matmul per 32-tile group applies W^T; DVE adds bias during the
    f32->f16 cast; SP streams idx tables in and fp16 results out; Activation
    streams the one-hot tables.
  - Host: inverse-permute per-core fp16 outputs to node order, cast fp32.
"""

import sys

import numpy as np

sys.path.insert(0, "/opt/trn_rl_repo")

N_NODES = 40000
N_EDGES = 640000
D = 128
P = 128
N_CORES = 8
LANES = 16  # nodes per tile
TILES_PER_CORE = 320
GROUP_TILES = 32  # tiles per psum-bank group (32*16 = 512 lanes)
GROUPS = TILES_PER_CORE // GROUP_TILES  # 10
SLOTS_PER_CORE = TILES_PER_CORE * LANES  # 5120
N_BINS = N_CORES * TILES_PER_CORE  # 2560 tiles globally
CAP = 2 * P  # 256 edges per tile (2 chunks x 128)
A_LIM = 32768  # A chunk: src < 32768 (idx = src)
B_OFF = 16384  # B chunk: src >= 16384 (idx = src - 16384)
SUB = 1024  # gather subcall size (hard real-SWDGE ring limit)
SCRATCH = 16384

_PROGRAM_CACHE: dict = {}


def _pack_nodes(degA, degB, deg):
    """Greedy pack nodes into N_BINS bins of <=16 nodes with per-bin caps:
    must-A edges (src<16384) <= 128, must-B (src>=32768) <= 128, total <= 256.
    Returns (node_bin, node_lane) or None."""
    import heapq

    order = np.argsort(-deg, kind="stable")
    heap = [(0.0, b) for b in range(N_BINS)]
    heapq.heapify(heap)
    cnt = np.zeros(N_BINS, dtype=np.int64)
    ldA = np.zeros(N_BINS, dtype=np.int64)
    ldB = np.zeros(N_BINS, dtype=np.int64)
    ldT = np.zeros(N_BINS, dtype=np.int64)
    node_bin = np.empty(N_NODES, dtype=np.int32)
    node_lane = np.empty(N_NODES, dtype=np.int32)
    for n in order:
        dA, dB, dT = int(degA[n]), int(degB[n]), int(deg[n])
        stash = []
        placed = False
        while heap:
            s, b = heapq.heappop(heap)
            if (
                cnt[b] < LANES
                and ldA[b] + dA <= P
                and ldB[b] + dB <= P
                and ldT[b] + dT <= CAP
            ):
                node_bin[n] = b
                node_lane[n] = cnt[b]
                cnt[b] += 1
                ldA[b] += dA
                ldB[b] += dB
                ldT[b] += dT
                ns = max(
                    (ldT[b] + dT) / CAP,
                    (ldA[b] + dA) / P,
                    (ldB[b] + dB) / P,
                    (cnt[b] + 1) / LANES,
                )
                stash.append((ns, b))
                placed = True
                break
            if cnt[b] < LANES:
                stash.append((s + 0.02, b))
        for item in stash:
            heapq.heappush(heap, item)
        if not placed:
            return None
    return node_bin, node_lane


def _wrap_idx(arr: np.ndarray) -> np.ndarray:
    """int16 flat idx list -> [128, len/16] wrapped + replicated layout."""
    w = arr.reshape(-1, 16).T  # [16, n/16]
    return np.ascontiguousarray(np.tile(w, (8, 1)))


def _prepare(x, src, dst, W, b):
    src = np.asarray(src).astype(np.int64)
    dst = np.asarray(dst).astype(np.int64)
    mustA = src < B_OFF  # must go in A chunk
    mustB = src >= A_LIM  # must go in B chunk
    degA = np.bincount(dst[mustA], minlength=N_NODES)
    degB = np.bincount(dst[mustB], minlength=N_NODES)
    deg = np.bincount(dst, minlength=N_NODES)

    packed = _pack_nodes(degA, degB, deg)
    assert packed is not None, "packing failed"
    node_bin, node_lane = packed
    # snake-deal bins to cores by load so per-core edge counts balance
    bin_load = np.zeros(N_BINS, dtype=np.int64)
    np.add.at(bin_load, node_bin[dst], 1)
    border = np.argsort(-bin_load, kind="stable")
    bin_core = np.empty(N_BINS, dtype=np.int64)
    bin_tile = np.empty(N_BINS, dtype=np.int64)
    for r, bb in enumerate(border.reshape(-1, N_CORES)):
        cores = range(N_CORES) if r % 2 == 0 else range(N_CORES - 1, -1, -1)
        for t, (c, bn) in enumerate(zip(cores, bb)):
            bin_core[bn] = c
            bin_tile[bn] = r

    # per-edge placement
    ebin = node_bin[dst]
    elane = node_lane[dst].astype(np.int64)
    ecore = bin_core[ebin]
    etile = bin_tile[ebin]
    # A/B assignment: mustB -> B; mustA -> A; flex fills A up to 128 then B.
    # Sort edges by (core, tile); assign per tile.
    eorder = np.lexsort((src, etile, ecore))
    s_src = src[eorder]
    s_lane = elane[eorder]
    s_core = ecore[eorder]
    s_tile = etile[eorder]

    key = s_core * TILES_PER_CORE + s_tile
    seg = np.searchsorted(key, np.arange(N_BINS + 1), side="left")

    # idx tables [core, group, unit(8 tiles), A/B, 8*128]; flat order per
    # group: A(tiles 0-7), B(0-7), A(8-15), B(8-15), ...
    UNIT = SUB // P  # 8 tiles per subcall
    idxs = np.zeros(
        (N_CORES, GROUPS, GROUP_TILES // UNIT, 2, UNIT * P), dtype=np.int16
    )
    oh = np.zeros(
        (N_CORES, P, GROUPS, 2 * GROUP_TILES, LANES), dtype=np.float16
    )
    for bn in range(N_BINS):
        c = None
        e0, e1 = seg[bn], seg[bn + 1]
        if e1 == e0:
            continue
        c = int(s_core[e0])
        t = int(s_tile[e0])
        g, ti = divmod(t, GROUP_TILES)
        esrc = s_src[e0:e1]
        elan = s_lane[e0:e1]
        isB_must = esrc >= A_LIM
        isA_must = esrc < B_OFF
        flex = ~isB_must & ~isA_must
        nA_must = int(isA_must.sum())
        nB_must = int(isB_must.sum())
        n = e1 - e0
        assert nA_must <= P and nB_must <= P and n <= CAP
        # fill A with must-A then flex until 128; rest to B
        a_take = min(P - nA_must, int(flex.sum()), n - nA_must - nB_must)
        # also ensure B fits: B gets nB_must + (flex - a_take) <= P
        b_cnt = n - nA_must - a_take
        if b_cnt > P:
            a_take += b_cnt - P
            b_cnt = P
        sel_flex = np.flatnonzero(flex)
        inA = np.concatenate([np.flatnonzero(isA_must), sel_flex[:a_take]])
        inB = np.concatenate([sel_flex[a_take:], np.flatnonzero(isB_must)])
        assert len(inA) <= P and len(inB) <= P
        unit, tw = divmod(ti, UNIT)
        idxs[c, g, unit, 0, tw * P : tw * P + len(inA)] = esrc[inA].astype(
            np.int16
        )
        idxs[c, g, unit, 1, tw * P : tw * P + len(inB)] = (
            esrc[inB] - B_OFF
        ).astype(np.int16)
        # one-hot: chunk index within group = 2*ti (A), 2*ti+1 (B)
        pa = np.arange(len(inA))
        oh[c, pa, g, 2 * ti, elan[inA]] = 1.0
        pb = np.arange(len(inB))
        oh[c, pb, g, 2 * ti + 1, elan[inB]] = 1.0

    xu = np.ascontiguousarray(np.asarray(x).astype(np.float16)).view(
        np.uint32
    )  # [40000, 64]
    wt = np.ascontiguousarray(np.asarray(W).T.astype(np.float16))
    bc = np.asarray(b).astype(np.float32)[:, None]  # [128,1]

    in_maps = []
    for c in range(N_CORES):
        in_maps.append(
            {
                "xu": xu,
                "idx": _wrap_idx(idxs[c].reshape(-1)),
                "oh": np.ascontiguousarray(
                    oh[c].reshape(P, GROUPS * 2 * GROUP_TILES * LANES)
                ),
                "wt": wt,
                "bc": bc,
            }
        )

    slot_node = np.full(N_BINS * LANES, -1, dtype=np.int64)
    gslot = (
        bin_core[node_bin] * SLOTS_PER_CORE
        + bin_tile[node_bin] * LANES
        + node_lane
    )
    slot_node[gslot] = np.arange(N_NODES)
    return in_maps, slot_node


def _build_program():
    import concourse.mybir as mybir
    import concourse.tile as tile
    from concourse import bacc

    f16 = mybir.dt.float16
    f32 = mybir.dt.float32
    u64 = mybir.dt.uint64
    i16 = mybir.dt.int16

    NIDX = GROUPS * 2 * GROUP_TILES * P  # 81920 per core
    OHW = GROUPS * 2 * GROUP_TILES * LANES  # one-hot cols

    u32 = mybir.dt.uint32
    nc = bacc.Bacc("TRN2", dynamic_dma_scratch_size=SCRATCH)
    xu = nc.dram_tensor("xu", [N_NODES, 64], u32, kind="ExternalInput")
    xa_v = xu[:A_LIM, :]  # A chunks gather rows < 32768
    xb_v = xu[B_OFF:, :]  # B chunks gather rows >= 16384 (idx = src-16384)
    idx_d = nc.dram_tensor("idx", [P, NIDX // 16], i16, kind="ExternalInput")
    oh_d = nc.dram_tensor("oh", [P, OHW], f16, kind="ExternalInput")
    wt_d = nc.dram_tensor("wt", [D, D], f16, kind="ExternalInput")
    bc_d = nc.dram_tensor("bc", [D, 1], f32, kind="ExternalInput")
    outT = nc.dram_tensor("outT", [P, SLOTS_PER_CORE], f16, kind="ExternalOutput")

    GIDX = 2 * GROUP_TILES * P  # idxs per group (A+B)

    SUBTILES = SUB // P  # 16 tiles' chunks per subcall

    with tile.TileContext(nc) as tc:
        with (
            tc.tile_pool(name="const", bufs=1) as cpool,
            tc.tile_pool(name="idxp", bufs=3) as idx_pool,
            tc.tile_pool(name="mg", bufs=12) as m_pool,
            tc.tile_pool(name="ohp", bufs=3) as oh_pool,
            tc.tile_pool(name="hsb", bufs=2) as h_pool,
            tc.tile_pool(name="osb", bufs=2) as o_pool,
            tc.tile_pool(name="ps1", bufs=2, space="PSUM") as ps1_pool,
            tc.tile_pool(name="ps2", bufs=2, space="PSUM") as ps2_pool,
        ):
            wt_t = cpool.tile([D, D], f16)
            nc.scalar.dma_start(out=wt_t[:], in_=wt_d[:])
            bc_t = cpool.tile([D, 1], f32)
            nc.scalar.dma_start(out=bc_t[:], in_=bc_d[:])

            for g in range(GROUPS):
                idx_t = idx_pool.tile([P, GIDX // 16], i16, tag="idxp")
                nc.sync.dma_start(
                    out=idx_t[:],
                    in_=idx_d[:, g * GIDX // 16 : (g + 1) * GIDX // 16],
                )
                # gather A+B rows (u32 rows, one tile per 2048-idx subcall so
                # matmuls start as soon as each subcall lands)
                # subcall order: A(tiles 0-7), B(0-7), A(8-15), B(8-15), ...
                subs = []
                for s in range(GIDX // SUB):
                    ms = m_pool.tile([P, SUBTILES, 64], u32, tag="mg", name=f"ms_{g}_{s}")
                    subs.append(ms)
                    nc.gpsimd.dma_gather(
                        out_ap=ms[:],
                        in_ap=xb_v if s % 2 == 1 else xa_v,
                        idxs_ap=idx_t[:, (s * SUB) // 16 : ((s + 1) * SUB) // 16],
                        num_idxs=SUB,
                        num_idxs_reg=SUB,
                        elem_size=64,
                        elem_step=64,
                    )
                oht = oh_pool.tile([P, 2 * GROUP_TILES, LANES], f16, tag="ohp")
                nc.scalar.dma_start(
                    out=oht[:],
                    in_=oh_d[:, g * 2 * GROUP_TILES * LANES : (g + 1) * 2 * GROUP_TILES * LANES],
                )
                ps1 = ps1_pool.tile([P, GROUP_TILES * LANES], f32, tag="ps1")
                # sequential A(start)/B(stop) pairs per tile (one pending psum
                # chain at a time within the bank)
                for ti in range(GROUP_TILES):
                    half, tw = divmod(ti, SUBTILES)
                    mfA = subs[2 * half][:].bitcast(f16)
                    mfB = subs[2 * half + 1][:].bitcast(f16)
                    nc.tensor.matmul(
                        out=ps1[:, ti * LANES : (ti + 1) * LANES],
                        lhsT=mfA[:, tw, :],
                        rhs=oht[:, 2 * ti, :],
                        start=True,
                        stop=False,
                    )
                    nc.tensor.matmul(
                        out=ps1[:, ti * LANES : (ti + 1) * LANES],
                        lhsT=mfB[:, tw, :],
                        rhs=oht[:, 2 * ti + 1, :],
                        start=False,
                        stop=True,
                    )
                hsb = h_pool.tile([P, GROUP_TILES * LANES], f16, tag="hsb")
                nc.vector.tensor_copy(out=hsb[:], in_=ps1[:])
                ps2 = ps2_pool.tile([P, GROUP_TILES * LANES], f32, tag="ps2")
                nc.tensor.matmul(
                    out=ps2[:], lhsT=wt_t[:], rhs=hsb[:], start=True, stop=True
                )
                osb = o_pool.tile([P, GROUP_TILES * LANES], f16, tag="osb")
                nc.vector.tensor_scalar(
                    out=osb[:],
                    in0=ps2[:],
                    scalar1=bc_t[:],
                    scalar2=None,
                    op0=mybir.AluOpType.add,
                )
                nc.sync.dma_start(
                    out=outT[
                        :,
                        g * GROUP_TILES * LANES : (g + 1) * GROUP_TILES * LANES,
                    ],
                    in_=osb[:],
                )

    nc.finalize()
    return nc


def get_program():
    if "v3" not in _PROGRAM_CACHE:
        _PROGRAM_CACHE["v3"] = _build_program()
    return _PROGRAM_CACHE["v3"]


def kernel(x, src, dst, W, b):
    from concourse.bass_utils import run_bass_kernel_spmd

    in_maps, slot_node = _prepare(x, src, dst, W, b)
    nc = get_program()
    res = run_bass_kernel_spmd(nc, in_maps, list(range(N_CORES)))

    full = np.empty((N_NODES, D), dtype=np.float32)
    for c in range(N_CORES):
        o = res.results[c]["outT"]  # [128 feat, 5120 slots] f16
        sn = slot_node[c * SLOTS_PER_CORE : (c + 1) * SLOTS_PER_CORE]
        valid = sn >= 0
        full[sn[valid]] = o[:, valid].T.astype(np.float32)
    return full
